# revision 28
# baseline (speedup 1.0000x reference)
"""Trainium2 Bass kernel for nn_CumulativeFFT.

out[b,t,d,k,c] = (1/sqrt(2048)) * cumsum_t( x[b,t,d] * tw[t,k,c] )

Sharding: 8 cores = batch(4) x time-half(2). Each core computes its
(1024, 256, 32, 2) output shard; the cross-half running offset is computed
on-device from an x_prev input (zeros for first-half cores, so the program
stays SPMD-uniform).

The HBM output shard is M-MAJOR (col = m*256 + d, m = k*2+c); the host
unshard does the cheap (t, m, d) -> (t, d, k, c) transpose. Converts write
contiguous 1024-col chunks and stores fire per PAIR of psum tiles (8 chunk
stores of 2048 cols per block instead of 2 half-block stores), so the
first store launches after two converts of block 0 instead of after all
16 (was the dominant ~25us fill stall of the d-major layout).

Per-core algorithm (T on 126-row blocks, m-major contribution layout):
  - Contributions C[s, m*256+d] = x[s,d] * tw[s,m] built by per-m
    tensor_scalar ops (per-partition scalar = twiddle column): m 0..39
    on DVE (4x mode, ~127ns/op), the rest on Pool/GPSIMD (~450ns/op;
    Pool cannot read PSUM so it cannot help converting). Block 0 shifts
    12 builds Pool->DVE so its C_b psum tiles aren't gated ~15us by
    Pool's serial build stream.
  - Causal cumsum via two PE matmuls per 1024-col psum tile (ring of 4)
    with a constant lhsT: rows s<126 = upper-tri ones, row 126 = all-ones
    "carry row" holding the running carry in bf16.
  - PSUM -> bf16 convert (x bf16(1/sqrt(2048))): 16 per block; DVE takes
    the LAST tiles {8,10,12,14,15} so its 5 converts form one contiguous
    batch right after its build stream and the scheduler can never hoist
    the next block's builds between a DVE convert and the psum-ring slot
    it frees; ACT streams the other 11 back-to-back.
  - Blocksums j=1..4 / j=5..8 write quarter-slices of two wide psum
    tiles, drained by ONE wide copy each (bsA on ACT in its pre-convert
    window; bsB on DVE, deprioritized past block-0's builds). The carry
    scan runs entirely in bf16 on DVE (fast-mode adds, no separate
    carry-row copies); the j=1 add runs at base priority so carry(1)'s
    DMA lands early.
  - Carry-row DMAs are emitted two blocks ahead of their stores in
    SP-queue order (an SP DMA's sem WAIT holds the queue).
  - The 16-row REM block is computed TRANSPOSED via 128 tiny matmuls
    (lhsT = 128-col C chunk, rhs = the same ut slice) into [128, 16]
    psum slabs: 2048 convert-cols + one contiguous store instead of a
    full block's 16384 cols; the host stitches the transpose.
  - Loads ordered x_prev/tw_prev (offset chain) -> x block 0 + twiddles
    (builds) -> the rest; PE warmup matmuls ramp the p-state meanwhile.

Steady state runs at the 360GB/s store roofline (11.47us/block, stores
gap-free); ACT ~11.4, DVE ~11.1, Pool ~10.8 per block. 117.3us total =
~10.5us fill ramp + 8x11.47 stores + ~6us rem tail.
"""

import math
import sys

import numpy as np

sys.path.insert(0, "/opt/trn_rl_repo")

import ml_dtypes

BF16 = ml_dtypes.bfloat16

B, T, D, K = 4, 2048, 256, 32
M2 = 2 * K            # 64 (k,c) pairs
MH = M2 // 2          # 32 m's per C half-tile
NCORES = 8
TH = T // 2           # 1024 time steps per core
TB = 126              # time-block rows (partitions 0..125; 126 = carry)
NFULL = TH // TB      # 8
REM = TH - NFULL * TB # 16
NBLK = NFULL + 1      # 9
NPREV = TH // 128     # 8 (128-row blocks of the other half, for the offset)
WID = M2 * D          # 16384 = m-major row width (col = m*D + d)
WH = MH * D           # 8192 = half-tile width
PS_FREE = 1024        # psum tile free width: 4 m's = 2 banks (ring depth 4)
NPS = WID // PS_FREE  # 16 psum tiles per block
NORM = float(np.float32(BF16(1.0 / math.sqrt(T))))
NWARM = 16            # PE warmup matmuls (ramp p-state during loads)
BD = 40               # builds on DVE (m 0..39); the rest on Pool

# Convert engine per psum tile: DVE takes the LAST tiles ({8,10,12,14,15})
# so its 5 converts form one contiguous batch right after its build stream
# and the scheduler can never hoist the next block's builds between a
# DVE convert and the psum-ring slot it frees (that hoist cost ~2.5us per
# block). ACT takes the first tiles, so the early chunk stores are gated
# only by ACT's back-to-back convert stream.
D_TILES_FULL = (8, 10, 12, 14, 15)

_prog = None


def _twiddles_np():
    n = np.arange(T, dtype=np.float32)
    k = np.arange(K, dtype=np.float32)
    ang = np.float32(-2.0 * math.pi / T) * np.outer(n, k)   # (T, K) f32
    tw = np.stack([np.cos(ang), np.sin(ang)], axis=-1)       # (T, K, 2)
    return tw.reshape(T, M2).astype(BF16)                    # m = k*2 + c


def _build_program():
    import concourse.bass as bass
    import concourse.tile as tile
    from concourse import bacc, mybir

    ts = bass.ts
    bf = mybir.dt.bfloat16
    f32 = mybir.dt.float32

    nc = bacc.Bacc(
        "TRN2", target_bir_lowering=False, debug=False, num_devices=NCORES
    )
    xo_h = nc.dram_tensor("x_own", [TH, D], bf, kind="ExternalInput")
    xp_h = nc.dram_tensor("x_prev", [TH, D], bf, kind="ExternalInput")
    two_h = nc.dram_tensor("tw_own", [128, NBLK * M2], bf, kind="ExternalInput")
    two32_h = nc.dram_tensor(
        "tw_own32", [128, NBLK * M2], f32, kind="ExternalInput"
    )
    twp_h = nc.dram_tensor("tw_prev", [128, NPREV * M2], bf, kind="ExternalInput")
    ut_h = nc.dram_tensor("ut", [128, TB + REM], bf, kind="ExternalInput")
    out_h = nc.dram_tensor("out", [TH, WID], bf, kind="ExternalOutput")
    # Rem block output, TRANSPOSED: remT[p, 16*c + r] = out[1008+r, 128*c + p].
    # The 16-row rem block pays column-bound converts like a full block
    # (16384 cols); transposing via 128 tiny matmuls shrinks that to 2048
    # convert-cols and one contiguous store. Host undoes the transpose.
    outr_h = nc.dram_tensor("out_remT", [128, REM * 128], bf, kind="ExternalOutput")

    with tile.TileContext(nc) as tc:
        with (
            tc.tile_pool(name="const", bufs=1) as cpool,
            tc.tile_pool(name="carry", bufs=1) as carpool,
            tc.tile_pool(name="cbuf", bufs=2) as cbpool,
            tc.tile_pool(name="obuf", bufs=3) as obpool,
            tc.tile_pool(name="ps", bufs=4, space="PSUM") as pspool,
        ):
            # PE warmup: ramp the p-state while input DMAs stream in.
            warm = cpool.tile([128, 512], bf, tag="warm")
            nc.gpsimd.memset(warm[:, :], 0)
            # Dummy ACT op: hoists the implicit 1.3us activation-table load
            # to t~0 so it isn't paid on the offset->carry(0) chain later.
            nc.scalar.copy(warm[0:1, 256:260], warm[0:1, 0:4])
            ps_w = pspool.tile([128, PS_FREE], f32, tag="ps")
            for _ in range(NWARM):
                nc.tensor.matmul(
                    ps_w[:, 0:128], warm[:, 0:128], warm[:, 0:128],
                    start=True, stop=True,
                )

            # Loads: x_prev/tw_prev first (they gate the offset->carry(0)
            # chain); x_own block 0 and the f32 twiddles next so DVE builds
            # start ASAP; the rest stream behind.
            xp_wide = cpool.tile([128, NPREV * D], bf, tag="xpw")
            nc.sync.dma_start(
                xp_wide[:, :],
                xp_h[:, :].rearrange("(i p) d -> p i d", p=128),
            )
            twp_t = cpool.tile([128, NPREV * M2], bf, tag="twp")
            nc.sync.dma_start(twp_t[:], twp_h[:])
            xo_wide = cpool.tile([128, NFULL * D], bf, tag="xow")
            nc.sync.dma_start(
                xo_wide[0:TB, 0:D],
                xo_h[0:TB, :],
            )
            two32_t = cpool.tile([128, NBLK * M2], f32, tag="two32")
            nc.sync.dma_start(two32_t[:], two32_h[:])
            two_t = cpool.tile([128, NBLK * M2], bf, tag="two")
            nc.sync.dma_start(two_t[:], two_h[:])
            ut_t = cpool.tile([128, TB + REM], bf, tag="ut")
            nc.sync.dma_start(ut_t[:], ut_h[:])
            nc.sync.dma_start(
                xo_wide[0:TB, D : 4 * D].rearrange("p (j d) -> p j d", d=D),
                xo_h[TB : 4 * TB, :].rearrange("(j p) d -> p j d", p=TB),
            )
            nc.sync.dma_start(
                xo_wide[0:TB, 4 * D :].rearrange("p (j d) -> p j d", d=D),
                xo_h[4 * TB : NFULL * TB, :].rearrange("(j p) d -> p j d", p=TB),
            )
            xo_rem = cpool.tile([128, D], bf, tag="xor")
            nc.sync.dma_start(xo_rem[0:REM, :], xo_h[NFULL * TB : TH, :])
            xo_tiles = [
                xo_wide[:, ts(j, D)] for j in range(NFULL)
            ] + [xo_rem[:, :]]
            xp_tiles = [xp_wide[:, ts(i, D)] for i in range(NPREV)]

            # ---- carry state ----
            # The whole carry scan runs in bf16 (one rounding per blocksum;
            # rel err stays well inside the 2e-2 gate): all-bf16 operands
            # put the DVE adds in fast mode and there is no separate
            # fp32->bf16 carry-row copy chain at all.
            carries = carpool.tile([64, NBLK * D], bf, tag="car")

            # Cross-half offset -> carries(0), on ACT so DVE's queue stays a
            # pure build stream at startup.
            ps_off = pspool.tile([64, D], f32, tag="ps")
            for i in range(NPREV):
                nc.tensor.matmul(
                    ps_off[:, :],
                    twp_t[:, ts(i, M2)],
                    xp_tiles[i],
                    start=(i == 0),
                    stop=(i == NPREV - 1),
                )
            nc.scalar.copy(carries[:, 0:D], ps_off[:, :])

            # ---- phase B: blocksums for ALL blocks, upfront, so every
            # carry row is ready early and the carry DMAs can be emitted two
            # blocks ahead of their stores on the serial SP queue.
            # Blocksums j=1..4 write quarter-slices of one wide psum tile,
            # j=5..8 another; each drains with ONE wide copy (no ping-pong
            # WAR stalls on the psum slots, and only ~2us of early copy
            # work). bsA's copy on ACT (idle pre-convert window); bsB's on
            # DVE, deprioritized past block-0's build stream (its carries
            # feed blocks 5..8, needed only after ~55us).
            bsram = carpool.tile([64, NFULL * D], bf, tag="bsram")
            bsA = pspool.tile([64, 4 * D], f32, tag="ps", name="bsA")
            bsB = pspool.tile([64, 4 * D], f32, tag="ps", name="bsB")
            prio_save = tc.cur_priority
            for j in range(1, NBLK):
                bs = bsA if j <= 4 else bsB
                nc.tensor.matmul(
                    bs[:, ts((j - 1) % 4, D)],
                    two_t[0:TB, ts(j - 1, M2)],
                    xo_tiles[j - 1][0:TB],
                    start=True,
                    stop=True,
                )
            nc.scalar.copy(bsram[:, 0 : 4 * D], bsA[:, :])
            tc.cur_priority = prio_save + 70
            nc.vector.tensor_copy(bsram[:, 4 * D :], bsB[:, :])
            tc.cur_priority = prio_save
            # add j=1 at base priority: it only needs bsA + the offset, so
            # carry(1)'s DMA can land ~11us instead of waiting for the whole
            # deprioritized scan; adds 2..8 follow off-peak.
            nc.vector.tensor_add(
                carries[:, ts(1, D)], carries[:, ts(0, D)], bsram[:, ts(0, D)]
            )
            tc.cur_priority = prio_save + 120
            for j in range(2, NBLK):
                nc.vector.tensor_add(
                    carries[:, ts(j, D)],
                    carries[:, ts(j - 1, D)],
                    bsram[:, ts(j - 1, D)],
                )
            tc.cur_priority = prio_save

            # C half-tiles rotate 2-deep via the pool; tiles for block j+2
            # are allocated during iteration j so their carry-row DMAs sit
            # ahead of stores(j+1) in SP-queue order.
            def alloc_c(j):
                C_a = cbpool.tile([128, WH], bf, tag="CA", name=f"CA{j}")
                C_b = cbpool.tile([128, WH], bf, tag="CB", name=f"CB{j}")
                return C_a, C_b

            def emit_carry_dma(j, C_a, C_b):
                ch = 126 if j < NFULL else REM
                for C_h, mbase in ((C_a, 0), (C_b, MH)):
                    nc.sync.dma_start(
                        C_h[ch : ch + 1, :].rearrange("p (a b) -> p a b", a=MH),
                        carries[mbase : mbase + MH, ts(j, D)],
                    )

            cqueue = []
            for j in range(2):
                Cn = alloc_c(j)
                emit_carry_dma(j, *Cn)
                cqueue.append(Cn)

            # ---- phase C: full blocks ----
            for j in range(NFULL):
                # Block 0 shifts 12 builds Pool->DVE: Pool's 10.8us build
                # stream would otherwise gate block 0's C_b psum tiles
                # (t10+) until ~15us, stretching the fill.
                bd = 52 if j == 0 else BD
                C_a, C_b = cqueue.pop(0)
                for mi in range(M2):
                    C_h = C_a if mi < MH else C_b
                    mh = mi if mi < MH else mi - MH
                    eng = nc.vector if mi < bd else nc.gpsimd
                    eng.tensor_scalar_mul(
                        C_h[0:TB, mh * D : (mh + 1) * D],
                        xo_tiles[j][0:TB],
                        two32_t[0:TB, j * M2 + mi : j * M2 + mi + 1],
                    )
                o_t = obpool.tile([128, WID], bf, tag="O")
                for n in range(NPS):
                    C_h = C_a if n < NPS // 2 else C_b
                    base = 0 if n < NPS // 2 else WH
                    ps_t = pspool.tile([128, PS_FREE], f32, tag="ps", name="ps")
                    for q in range(PS_FREE // 512):
                        col = n * PS_FREE + q * 512 - base
                        nc.tensor.matmul(
                            ps_t[:TB, ts(q, 512)],
                            ut_t[0:127, 0:TB],
                            C_h[0:127, col : col + 512],
                            start=True,
                            stop=True,
                        )
                    # psum -> bf16 convert with 1/sqrt(T) scale; contiguous
                    # m-major dst.
                    dst = o_t[:TB, n * PS_FREE : (n + 1) * PS_FREE]
                    if n in D_TILES_FULL:
                        nc.vector.tensor_scalar_mul(dst, ps_t[:TB, :], NORM)
                    else:
                        nc.scalar.mul(dst, ps_t[:TB, :], NORM)
                    # chunk store per pair of converted tiles (blocks 2+;
                    # blocks 0-1 store single tiles in completion order
                    # below, so the fill DMA never waits on the later of
                    # two converts)
                    if j >= 2 and n % 2 == 1:
                        nc.sync.dma_start(
                            out_h[
                                j * TB : j * TB + TB,
                                (n - 1) * PS_FREE : (n + 1) * PS_FREE,
                            ],
                            o_t[:TB, (n - 1) * PS_FREE : (n + 1) * PS_FREE],
                        )
                if j < 2:
                    # Single-tile stores in predicted convert-completion
                    # order: ACT streams t0..t7,t9,11,13 while DVE's
                    # t8,10,12,14,15 finish interleaved after its builds.
                    # The SP queue's head-of-line wait then tracks the
                    # actual convert stream instead of pair order.
                    for n in (0, 1, 8, 2, 10, 3, 12, 4, 14, 5, 6, 15, 7, 9, 11, 13):
                        nc.sync.dma_start(
                            out_h[
                                j * TB : j * TB + TB,
                                n * PS_FREE : (n + 1) * PS_FREE,
                            ],
                            o_t[:TB, n * PS_FREE : (n + 1) * PS_FREE],
                        )
                # carry rows for block j+2: ahead of stores(j+1) in SP order,
                # so they land ~a block before block j+2's matmuls read them.
                if j + 2 < NBLK:
                    Cn = alloc_c(j + 2)
                    emit_carry_dma(j + 2, *Cn)
                    cqueue.append(Cn)

            # ---- phase C: rem block (16 rows), TRANSPOSED ----
            # 128 tiny matmuls with the C chunk as the STATIONARY side:
            # psT[128c, r] = sum_s C[s, 128*i+c] * ut[s, r] -- the same
            # upper-tri+carry ut slice, just used as the moving operand.
            # Only 2048 psum cols to convert (2 ops) instead of 16384 (16),
            # and one contiguous store; the host undoes the transpose.
            C_a, C_b = cqueue.pop(0)
            for mi in range(M2):
                C_h = C_a if mi < MH else C_b
                mh = mi if mi < MH else mi - MH
                eng = nc.vector if mi < BD else nc.gpsimd
                eng.tensor_scalar_mul(
                    C_h[0:REM, mh * D : (mh + 1) * D],
                    xo_tiles[NFULL][0:REM],
                    two32_t[0:REM, NFULL * M2 + mi : NFULL * M2 + mi + 1],
                )
            o_rt = obpool.tile([128, 2 * PS_FREE], bf, tag="O", name="o_remT")
            for half, C_h in ((0, C_a), (1, C_b)):
                ps_t = pspool.tile([128, PS_FREE], f32, tag="ps", name="ps")
                for c in range(PS_FREE // REM):
                    nc.tensor.matmul(
                        ps_t[:, c * REM : (c + 1) * REM],
                        C_h[0 : REM + 1, c * 128 : (c + 1) * 128],
                        ut_t[0 : REM + 1, TB : TB + REM],
                        start=True,
                        stop=True,
                    )
                dst = o_rt[:, half * PS_FREE : (half + 1) * PS_FREE]
                if half == 0:
                    nc.scalar.mul(dst, ps_t[:, :], NORM)
                else:
                    nc.vector.tensor_scalar_mul(dst, ps_t[:, :], NORM)
            nc.sync.dma_start(outr_h[:, :], o_rt[:, :])
    nc.compile()
    return nc


def _host_inputs(x):
    tw = _twiddles_np()
    ut = np.zeros((128, TB + REM), dtype=BF16)
    ut[0:TB, 0:TB] = np.triu(np.ones((TB, TB), dtype=np.float32)).astype(BF16)
    ut[126:128, 0:TB] = 1
    ut[0:REM, TB : TB + REM] = np.triu(np.ones((REM, REM), dtype=np.float32)).astype(
        BF16
    )
    ut[REM : REM + 2, TB : TB + REM] = 1
    twp = np.zeros((128, NPREV * M2), dtype=BF16)
    for i in range(NPREV):
        twp[:, i * M2 : (i + 1) * M2] = tw[i * 128 : (i + 1) * 128, :]
    in_maps = []
    for c in range(NCORES):
        b, h = divmod(c, 2)
        base = h * TH
        xo = np.ascontiguousarray(x[b, base : base + TH, :])
        xp = (
            np.ascontiguousarray(x[b, 0:TH, :])
            if h
            else np.zeros((TH, D), dtype=BF16)
        )
        two = np.zeros((128, NBLK * M2), dtype=BF16)
        for j in range(NBLK):
            rows = TB if j < NFULL else REM
            two[0:rows, j * M2 : (j + 1) * M2] = tw[
                base + j * TB : base + j * TB + rows, :
            ]
        in_maps.append(
            {
                "x_own": xo,
                "x_prev": xp,
                "tw_own": two,
                "tw_own32": two.astype(np.float32),
                "tw_prev": twp,
                "ut": ut,
            }
        )
    return in_maps


def kernel(x):
    global _prog
    x = np.asarray(x)
    assert x.shape == (B, T, D), x.shape
    if x.dtype != BF16:
        x = x.astype(BF16)
    if _prog is None:
        _prog = _build_program()
    from concourse.bass_utils import run_bass_kernel_spmd

    in_maps = _host_inputs(x)
    res = run_bass_kernel_spmd(_prog, in_maps, list(range(NCORES)))
    out = np.empty((B, T, D, K, 2), dtype=BF16)
    for c in range(NCORES):
        b, h = divmod(c, 2)
        # HBM shard is m-major: col = (k*2+c)*D + d; the 16-row rem block
        # arrives transposed (remT[p, 16*ch + r] = row 1008+r, col
        # 128*ch + p). Stitch, then the cheap transpose to (t, d, k, c).
        shard_m = np.empty((TH, WID), dtype=BF16)
        shard_m[0 : NFULL * TB] = res.results[c]["out"][0 : NFULL * TB]
        remT = res.results[c]["out_remT"].reshape(128, WID // 128, REM)
        shard_m[NFULL * TB :] = remT.transpose(2, 1, 0).reshape(REM, WID)
        shard = shard_m.reshape(TH, K, 2, D)
        out[b, h * TH : (h + 1) * TH] = shard.transpose(0, 3, 1, 2)
    return out


# revision 29
# speedup vs baseline: 1.0330x; 1.0330x over previous
"""Trainium2 Bass kernel for nn_CumulativeFFT.

out[b,t,d,k,c] = (1/sqrt(2048)) * cumsum_t( x[b,t,d] * tw[t,k,c] )

Sharding: 8 cores = batch(4) x time-half(2). Each core computes its
(1024, 256, 32, 2) output shard; the cross-half running offset is computed
on-device from an x_prev input (zeros for first-half cores, so the program
stays SPMD-uniform).

The HBM output shard is M-MAJOR (col = m*256 + d, m = k*2+c); the host
unshard does the cheap (t, m, d) -> (t, d, k, c) transpose. Converts write
contiguous 1024-col chunks and stores fire per PAIR of psum tiles (8 chunk
stores of 2048 cols per block instead of 2 half-block stores), so the
first store launches after two converts of block 0 instead of after all
16 (was the dominant ~25us fill stall of the d-major layout).

Per-core algorithm (T on 126-row blocks, m-major contribution layout):
  - Contributions C[s, m*256+d] = x[s,d] * tw[s,m] built by per-m
    tensor_scalar ops (per-partition scalar = twiddle column): m 0..39
    on DVE (4x mode, ~127ns/op), the rest on Pool/GPSIMD (~450ns/op;
    Pool cannot read PSUM so it cannot help converting). Block 0 shifts
    12 builds Pool->DVE so its C_b psum tiles aren't gated ~15us by
    Pool's serial build stream.
  - Causal cumsum via two PE matmuls per 1024-col psum tile (ring of 4)
    with a constant lhsT: rows s<126 = upper-tri ones, row 126 = all-ones
    "carry row" holding the running carry in bf16.
  - PSUM -> bf16 convert (x bf16(1/sqrt(2048))): 16 per block; DVE takes
    the LAST tiles {8,10,12,14,15} so its 5 converts form one contiguous
    batch right after its build stream and the scheduler can never hoist
    the next block's builds between a DVE convert and the psum-ring slot
    it frees; ACT streams the other 11 back-to-back.
  - Blocksums j=1..4 / j=5..8 write quarter-slices of two wide psum
    tiles, drained by ONE wide copy each (bsA on ACT in its pre-convert
    window; bsB on DVE, deprioritized past block-0's builds). The carry
    scan runs entirely in bf16 on DVE (fast-mode adds, no separate
    carry-row copies); the j=1 add runs at base priority so carry(1)'s
    DMA lands early.
  - Carry-row DMAs are emitted two blocks ahead of their stores in
    SP-queue order (an SP DMA's sem WAIT holds the queue).
  - The 16-row REM block is computed TRANSPOSED via 128 tiny matmuls
    (lhsT = 128-col C chunk, rhs = the same ut slice) into [128, 16]
    psum slabs: 2048 convert-cols + one contiguous store instead of a
    full block's 16384 cols; the host stitches the transpose.
  - Loads ordered x_prev/tw_prev (offset chain) -> x block 0 + twiddles
    (builds) -> the rest; PE warmup matmuls ramp the p-state meanwhile.

Steady state runs at the 360GB/s store roofline (11.47us/block, stores
gap-free); ACT ~11.4, DVE ~11.1, Pool ~10.8 per block. 117.3us total =
~10.5us fill ramp + 8x11.47 stores + ~6us rem tail.
"""

import math
import sys

import numpy as np

sys.path.insert(0, "/opt/trn_rl_repo")

import ml_dtypes

BF16 = ml_dtypes.bfloat16

B, T, D, K = 4, 2048, 256, 32
M2 = 2 * K            # 64 (k,c) pairs
MH = M2 // 2          # 32 m's per C half-tile
NCORES = 8
TH = T // 2           # 1024 time steps per core
TB = 126              # time-block rows (partitions 0..125; 126 = carry)
NFULL = TH // TB      # 8
REM = TH - NFULL * TB # 16
NBLK = NFULL + 1      # 9
NPREV = TH // 128     # 8 (128-row blocks of the other half, for the offset)
WID = M2 * D          # 16384 = m-major row width (col = m*D + d)
WH = MH * D           # 8192 = half-tile width
PS_FREE = 1024        # psum tile free width: 4 m's = 2 banks (ring depth 4)
NPS = WID // PS_FREE  # 16 psum tiles per block
NORM = float(np.float32(BF16(1.0 / math.sqrt(T))))
NWARM = 16            # PE warmup matmuls (ramp p-state during loads)
BD = 40               # builds on DVE (m 0..39); the rest on Pool

# Convert engine per psum tile: DVE takes the LAST tiles ({8,10,12,14,15})
# so its 5 converts form one contiguous batch right after its build stream
# and the scheduler can never hoist the next block's builds between a
# DVE convert and the psum-ring slot it frees (that hoist cost ~2.5us per
# block). ACT takes the first tiles, so the early chunk stores are gated
# only by ACT's back-to-back convert stream.
D_TILES_FULL = (8, 10, 12, 14, 15)

_prog = None


def _twiddles_np():
    n = np.arange(T, dtype=np.float32)
    k = np.arange(K, dtype=np.float32)
    ang = np.float32(-2.0 * math.pi / T) * np.outer(n, k)   # (T, K) f32
    tw = np.stack([np.cos(ang), np.sin(ang)], axis=-1)       # (T, K, 2)
    return tw.reshape(T, M2).astype(BF16)                    # m = k*2 + c


def _build_program():
    import concourse.bass as bass
    import concourse.tile as tile
    from concourse import bacc, mybir

    ts = bass.ts
    bf = mybir.dt.bfloat16
    f32 = mybir.dt.float32

    nc = bacc.Bacc(
        "TRN2", target_bir_lowering=False, debug=False, num_devices=NCORES
    )
    xo_h = nc.dram_tensor("x_own", [TH, D], bf, kind="ExternalInput")
    xp_h = nc.dram_tensor("x_prev", [TH, D], bf, kind="ExternalInput")
    two_h = nc.dram_tensor("tw_own", [128, NBLK * M2], bf, kind="ExternalInput")
    two32_h = nc.dram_tensor(
        "tw_own32", [128, NBLK * M2], f32, kind="ExternalInput"
    )
    twp_h = nc.dram_tensor("tw_prev", [128, NPREV * M2], bf, kind="ExternalInput")
    ut_h = nc.dram_tensor("ut", [128, TB + REM], bf, kind="ExternalInput")
    out_h = nc.dram_tensor("out", [TH, WID], bf, kind="ExternalOutput")
    # Rem block output, TRANSPOSED: remT[p, 16*c + r] = out[1008+r, 128*c + p].
    # The 16-row rem block pays column-bound converts like a full block
    # (16384 cols); transposing via 128 tiny matmuls shrinks that to 2048
    # convert-cols and one contiguous store. Host undoes the transpose.
    outr_h = nc.dram_tensor("out_remT", [128, REM * 128], bf, kind="ExternalOutput")

    with tile.TileContext(nc) as tc:
        with (
            tc.tile_pool(name="const", bufs=1) as cpool,
            tc.tile_pool(name="carry", bufs=1) as carpool,
            tc.tile_pool(name="cbuf", bufs=2) as cbpool,
            tc.tile_pool(name="obuf", bufs=3) as obpool,
            tc.tile_pool(name="ps", bufs=4, space="PSUM") as pspool,
        ):
            # PE warmup: ramp the p-state while input DMAs stream in.
            warm = cpool.tile([128, 512], bf, tag="warm")
            nc.gpsimd.memset(warm[:, :], 0)
            # Dummy ACT op: hoists the implicit 1.3us activation-table load
            # to t~0 so it isn't paid on the offset->carry(0) chain later.
            nc.scalar.copy(warm[0:1, 256:260], warm[0:1, 0:4])
            ps_w = pspool.tile([128, PS_FREE], f32, tag="ps")
            for _ in range(NWARM):
                nc.tensor.matmul(
                    ps_w[:, 0:128], warm[:, 0:128], warm[:, 0:128],
                    start=True, stop=True,
                )

            # Loads: x_prev/tw_prev first (they gate the offset->carry(0)
            # chain); x_own block 0 and the f32 twiddles next so DVE builds
            # start ASAP; the rest stream behind.
            xp_wide = cpool.tile([128, NPREV * D], bf, tag="xpw")
            nc.sync.dma_start(
                xp_wide[:, :],
                xp_h[:, :].rearrange("(i p) d -> p i d", p=128),
            )
            twp_t = cpool.tile([128, NPREV * M2], bf, tag="twp")
            nc.sync.dma_start(twp_t[:], twp_h[:])
            xo_wide = cpool.tile([128, NFULL * D], bf, tag="xow")
            nc.sync.dma_start(
                xo_wide[0:TB, 0:D],
                xo_h[0:TB, :],
            )
            two32_t = cpool.tile([128, NBLK * M2], f32, tag="two32")
            nc.sync.dma_start(two32_t[:], two32_h[:])
            two_t = cpool.tile([128, NBLK * M2], bf, tag="two")
            nc.sync.dma_start(two_t[:], two_h[:])
            ut_t = cpool.tile([128, TB + REM], bf, tag="ut")
            nc.sync.dma_start(ut_t[:], ut_h[:])
            nc.sync.dma_start(
                xo_wide[0:TB, D : 4 * D].rearrange("p (j d) -> p j d", d=D),
                xo_h[TB : 4 * TB, :].rearrange("(j p) d -> p j d", p=TB),
            )
            nc.sync.dma_start(
                xo_wide[0:TB, 4 * D :].rearrange("p (j d) -> p j d", d=D),
                xo_h[4 * TB : NFULL * TB, :].rearrange("(j p) d -> p j d", p=TB),
            )
            xo_rem = cpool.tile([128, D], bf, tag="xor")
            nc.sync.dma_start(xo_rem[0:REM, :], xo_h[NFULL * TB : TH, :])
            xo_tiles = [
                xo_wide[:, ts(j, D)] for j in range(NFULL)
            ] + [xo_rem[:, :]]
            xp_tiles = [xp_wide[:, ts(i, D)] for i in range(NPREV)]

            # ---- carry state ----
            # The whole carry scan runs in bf16 (one rounding per blocksum;
            # rel err stays well inside the 2e-2 gate): all-bf16 operands
            # put the DVE adds in fast mode and there is no separate
            # fp32->bf16 carry-row copy chain at all.
            carries = carpool.tile([64, NBLK * D], bf, tag="car")

            # Cross-half offset -> carries(0), on ACT so DVE's queue stays a
            # pure build stream at startup.
            ps_off = pspool.tile([64, D], f32, tag="ps")
            for i in range(NPREV):
                nc.tensor.matmul(
                    ps_off[:, :],
                    twp_t[:, ts(i, M2)],
                    xp_tiles[i],
                    start=(i == 0),
                    stop=(i == NPREV - 1),
                )
            nc.scalar.copy(carries[:, 0:D], ps_off[:, :])

            # ---- phase B: blocksums for ALL blocks, upfront, so every
            # carry row is ready early and the carry DMAs can be emitted two
            # blocks ahead of their stores on the serial SP queue.
            # Blocksums j=1..4 write quarter-slices of one wide psum tile,
            # j=5..8 another; each drains with ONE wide copy (no ping-pong
            # WAR stalls on the psum slots, and only ~2us of early copy
            # work). bsA's copy on ACT (idle pre-convert window); bsB's on
            # DVE, deprioritized past block-0's build stream (its carries
            # feed blocks 5..8, needed only after ~55us).
            bsram = carpool.tile([64, NFULL * D], bf, tag="bsram")
            bsA = pspool.tile([64, 4 * D], f32, tag="ps", name="bsA")
            bsB = pspool.tile([64, 4 * D], f32, tag="ps", name="bsB")
            prio_save = tc.cur_priority
            for j in range(1, NBLK):
                bs = bsA if j <= 4 else bsB
                nc.tensor.matmul(
                    bs[:, ts((j - 1) % 4, D)],
                    two_t[0:TB, ts(j - 1, M2)],
                    xo_tiles[j - 1][0:TB],
                    start=True,
                    stop=True,
                )
            nc.scalar.copy(bsram[:, 0 : 4 * D], bsA[:, :])
            tc.cur_priority = prio_save + 70
            nc.vector.tensor_copy(bsram[:, 4 * D :], bsB[:, :])
            tc.cur_priority = prio_save
            # add j=1 at base priority: it only needs bsA + the offset, so
            # carry(1)'s DMA can land ~11us instead of waiting for the whole
            # deprioritized scan; adds 2..8 follow off-peak.
            nc.vector.tensor_add(
                carries[:, ts(1, D)], carries[:, ts(0, D)], bsram[:, ts(0, D)]
            )
            tc.cur_priority = prio_save + 120
            for j in range(2, NBLK):
                nc.vector.tensor_add(
                    carries[:, ts(j, D)],
                    carries[:, ts(j - 1, D)],
                    bsram[:, ts(j - 1, D)],
                )
            tc.cur_priority = prio_save

            # C half-tiles rotate 2-deep via the pool; tiles for block j+2
            # are allocated during iteration j so their carry-row DMAs sit
            # ahead of stores(j+1) in SP-queue order.
            def alloc_c(j):
                C_a = cbpool.tile([128, WH], bf, tag="CA", name=f"CA{j}")
                C_b = cbpool.tile([128, WH], bf, tag="CB", name=f"CB{j}")
                return C_a, C_b

            def emit_carry_dma(j, C_a, C_b):
                ch = 126 if j < NFULL else REM
                for C_h, mbase in ((C_a, 0), (C_b, MH)):
                    nc.sync.dma_start(
                        C_h[ch : ch + 1, :].rearrange("p (a b) -> p a b", a=MH),
                        carries[mbase : mbase + MH, ts(j, D)],
                    )

            cqueue = []
            for j in range(2):
                Cn = alloc_c(j)
                emit_carry_dma(j, *Cn)
                cqueue.append(Cn)

            # ---- phase C: full blocks ----
            for j in range(NFULL):
                # Block 0 shifts 12 builds Pool->DVE: Pool's 10.8us build
                # stream would otherwise gate block 0's C_b psum tiles
                # (t10+) until ~15us, stretching the fill.
                bd = 52 if j == 0 else BD
                C_a, C_b = cqueue.pop(0)
                for mi in range(M2):
                    C_h = C_a if mi < MH else C_b
                    mh = mi if mi < MH else mi - MH
                    eng = nc.vector if mi < bd else nc.gpsimd
                    eng.tensor_scalar_mul(
                        C_h[0:TB, mh * D : (mh + 1) * D],
                        xo_tiles[j][0:TB],
                        two32_t[0:TB, j * M2 + mi : j * M2 + mi + 1],
                    )
                o_t = obpool.tile([128, WID], bf, tag="O")
                for n in range(NPS):
                    C_h = C_a if n < NPS // 2 else C_b
                    base = 0 if n < NPS // 2 else WH
                    ps_t = pspool.tile([128, PS_FREE], f32, tag="ps", name="ps")
                    for q in range(PS_FREE // 512):
                        col = n * PS_FREE + q * 512 - base
                        nc.tensor.matmul(
                            ps_t[:TB, ts(q, 512)],
                            ut_t[0:127, 0:TB],
                            C_h[0:127, col : col + 512],
                            start=True,
                            stop=True,
                        )
                    # psum -> bf16 convert with 1/sqrt(T) scale; contiguous
                    # m-major dst.
                    dst = o_t[:TB, n * PS_FREE : (n + 1) * PS_FREE]
                    if n in D_TILES_FULL:
                        nc.vector.tensor_scalar_mul(dst, ps_t[:TB, :], NORM)
                    else:
                        nc.scalar.mul(dst, ps_t[:TB, :], NORM)
                    # chunk store per pair of converted tiles
                    if n % 2 == 1:
                        nc.sync.dma_start(
                            out_h[
                                j * TB : j * TB + TB,
                                (n - 1) * PS_FREE : (n + 1) * PS_FREE,
                            ],
                            o_t[:TB, (n - 1) * PS_FREE : (n + 1) * PS_FREE],
                        )
                # carry rows for block j+2: ahead of stores(j+1) in SP order,
                # so they land ~a block before block j+2's matmuls read them.
                if j + 2 < NBLK:
                    Cn = alloc_c(j + 2)
                    emit_carry_dma(j + 2, *Cn)
                    cqueue.append(Cn)

            # ---- phase C: rem block (16 rows), TRANSPOSED ----
            # 128 tiny matmuls with the C chunk as the STATIONARY side:
            # psT[128c, r] = sum_s C[s, 128*i+c] * ut[s, r] -- the same
            # upper-tri+carry ut slice, just used as the moving operand.
            # Only 2048 psum cols to convert (2 ops) instead of 16384 (16),
            # and one contiguous store; the host undoes the transpose.
            C_a, C_b = cqueue.pop(0)
            for mi in range(M2):
                C_h = C_a if mi < MH else C_b
                mh = mi if mi < MH else mi - MH
                eng = nc.vector if mi < BD else nc.gpsimd
                eng.tensor_scalar_mul(
                    C_h[0:REM, mh * D : (mh + 1) * D],
                    xo_tiles[NFULL][0:REM],
                    two32_t[0:REM, NFULL * M2 + mi : NFULL * M2 + mi + 1],
                )
            o_rt = obpool.tile([128, 2 * PS_FREE], bf, tag="O", name="o_remT")
            for half, C_h in ((0, C_a), (1, C_b)):
                ps_t = pspool.tile([128, PS_FREE], f32, tag="ps", name="ps")
                for c in range(PS_FREE // REM):
                    nc.tensor.matmul(
                        ps_t[:, c * REM : (c + 1) * REM],
                        C_h[0 : REM + 1, c * 128 : (c + 1) * 128],
                        ut_t[0 : REM + 1, TB : TB + REM],
                        start=True,
                        stop=True,
                    )
                dst = o_rt[:, half * PS_FREE : (half + 1) * PS_FREE]
                if half == 0:
                    nc.scalar.mul(dst, ps_t[:, :], NORM)
                else:
                    nc.vector.tensor_scalar_mul(dst, ps_t[:, :], NORM)
            nc.sync.dma_start(outr_h[:, :], o_rt[:, :])
    nc.compile()
    return nc


def _host_inputs(x):
    tw = _twiddles_np()
    ut = np.zeros((128, TB + REM), dtype=BF16)
    ut[0:TB, 0:TB] = np.triu(np.ones((TB, TB), dtype=np.float32)).astype(BF16)
    ut[126:128, 0:TB] = 1
    ut[0:REM, TB : TB + REM] = np.triu(np.ones((REM, REM), dtype=np.float32)).astype(
        BF16
    )
    ut[REM : REM + 2, TB : TB + REM] = 1
    twp = np.zeros((128, NPREV * M2), dtype=BF16)
    for i in range(NPREV):
        twp[:, i * M2 : (i + 1) * M2] = tw[i * 128 : (i + 1) * 128, :]
    in_maps = []
    for c in range(NCORES):
        b, h = divmod(c, 2)
        base = h * TH
        xo = np.ascontiguousarray(x[b, base : base + TH, :])
        xp = (
            np.ascontiguousarray(x[b, 0:TH, :])
            if h
            else np.zeros((TH, D), dtype=BF16)
        )
        two = np.zeros((128, NBLK * M2), dtype=BF16)
        for j in range(NBLK):
            rows = TB if j < NFULL else REM
            two[0:rows, j * M2 : (j + 1) * M2] = tw[
                base + j * TB : base + j * TB + rows, :
            ]
        in_maps.append(
            {
                "x_own": xo,
                "x_prev": xp,
                "tw_own": two,
                "tw_own32": two.astype(np.float32),
                "tw_prev": twp,
                "ut": ut,
            }
        )
    return in_maps


def kernel(x):
    global _prog
    x = np.asarray(x)
    assert x.shape == (B, T, D), x.shape
    if x.dtype != BF16:
        x = x.astype(BF16)
    if _prog is None:
        _prog = _build_program()
    from concourse.bass_utils import run_bass_kernel_spmd

    in_maps = _host_inputs(x)
    res = run_bass_kernel_spmd(_prog, in_maps, list(range(NCORES)))
    out = np.empty((B, T, D, K, 2), dtype=BF16)
    for c in range(NCORES):
        b, h = divmod(c, 2)
        # HBM shard is m-major: col = (k*2+c)*D + d; the 16-row rem block
        # arrives transposed (remT[p, 16*ch + r] = row 1008+r, col
        # 128*ch + p). Stitch, then the cheap transpose to (t, d, k, c).
        shard_m = np.empty((TH, WID), dtype=BF16)
        shard_m[0 : NFULL * TB] = res.results[c]["out"][0 : NFULL * TB]
        remT = res.results[c]["out_remT"].reshape(128, WID // 128, REM)
        shard_m[NFULL * TB :] = remT.transpose(2, 1, 0).reshape(REM, WID)
        shard = shard_m.reshape(TH, K, 2, D)
        out[b, h * TH : (h + 1) * TH] = shard.transpose(0, 3, 1, 2)
    return out
